# revision 4
# baseline (speedup 1.0000x reference)
"""Causal multi-head self-attention on 8 Trainium2 NeuronCores.

Sharding: core c = (b, g) with b = c // 4 (batch), g = c % 4 (head group).
Each core computes 4 of the 16 heads for one batch element:
  Q/K/V projections for feature rows 256g:256g+256 (Megatron column split),
  causal attention for those heads, and a partial output projection
  against Wo[:, 256g:256g+256] (row split). Host sums the 4 partials per batch.

All operands are pre-shuffled on the host so every DMA is contiguous per
partition (>=1KB lines) and the kernel never transposes:
  xt  = X[b].T    reshaped to [P, NSB, KC, SB]  (d on partitions)
  wqt = Wq[rows].T reshaped to [P, KC, F]       (contraction chunk-major)
  ... same for wk, wv; wot = Wo[:, cols].T as [P, MC, D].

Projections and attention are interleaved per 512-token block: attention for
q-block qb only needs K/V blocks 0..qb, so it runs right after block qb's
projections while the next x block streams in.  Attention keeps scores
transposed (S^T = K Q^T, kv on partitions) so PV needs no transpose, and a
ones-row appended to V yields the softmax denominator inside the same psum
accumulation.  The causal structure is exploited at 128-column granularity:
score/exp/PV work below the diagonal band is skipped, and the in-band
triangle is zeroed with a gpsimd affine_select on the exp output.
"""

import sys

sys.path.insert(0, "/opt/trn_rl_repo")

import numpy as np

B = 2
S = 2048
D = 1024
H = 16
DH = 64

NCORES = 8
GROUPS = 4            # head groups (cores per batch element)
HPC = H // GROUPS     # heads per core = 4
F = HPC * DH          # feature slice per core = 256

_nc_cache = {}


def _build(s=S):
    import concourse.bass as bass  # noqa: F401
    import concourse.mybir as mybir
    import concourse.tile as tile
    from concourse import bacc

    f32 = mybir.dt.float32
    f16 = mybir.dt.float16
    bf16 = mybir.dt.bfloat16
    dmm = bf16  # matmul operand dtype

    P = 128
    SB = 512               # q-block / free-dim block
    NSB = s // SB          # q blocks
    KC = D // P            # 8 contraction chunks over D
    MC = F // P            # 2 feature chunks per core
    NSC = s // P           # s chunks of 128
    ND = D // SB           # 2 output column blocks

    nc = bacc.Bacc("TRN2", debug=False, num_devices=NCORES)
    xt = nc.dram_tensor("xt", [P, NSB, KC, SB], dmm, kind="ExternalInput").ap()
    wqt = nc.dram_tensor("wqt", [P, KC, F], dmm, kind="ExternalInput").ap()
    wkt = nc.dram_tensor("wkt", [P, KC, F], dmm, kind="ExternalInput").ap()
    wvt = nc.dram_tensor("wvt", [P, KC, F], dmm, kind="ExternalInput").ap()
    wot = nc.dram_tensor("wot", [P, MC, D], dmm, kind="ExternalInput").ap()
    y = nc.dram_tensor("y", [s, D], f16, kind="ExternalOutput").ap()

    with tile.TileContext(nc) as tc:
        with (
            tc.tile_pool(name="w", bufs=1) as wpool,
            tc.tile_pool(name="const", bufs=1) as cpool,
            tc.tile_pool(name="xt", bufs=2) as xpool,
            tc.tile_pool(name="qkv", bufs=1) as qkvpool,
            tc.tile_pool(name="pt", bufs=3) as ptpool,
            tc.tile_pool(name="small", bufs=2) as spool,
            tc.tile_pool(name="yst", bufs=3) as ypool,
            tc.tile_pool(name="ps", bufs=1, space="PSUM") as pspool,
        ):
            # --- weights ---
            wq_s = wpool.tile([P, KC, F], dmm, name="wq_s")
            wk_s = wpool.tile([P, KC, F], dmm, name="wk_s")
            wv_s = wpool.tile([P, KC, F], dmm, name="wv_s")
            wo_s = wpool.tile([P, MC, D], dmm, name="wo_s")

            # --- persistent activations ---
            qt_t = qkvpool.tile([P, MC, s], dmm, name="qt_t")   # Q^T
            kt_t = qkvpool.tile([P, MC, s], dmm, name="kt_t")   # K^T
            v_t = qkvpool.tile([P, NSC, HPC, DH + 1], dmm, name="v_t")  # V | 1
            ot_t = qkvpool.tile([P, MC, s], dmm, name="ot_t")   # attn out ^T

            # DMA order: wq -> x0 (per-chunk) -> wk -> wv -> x1 -> wo -> x2,x3
            nc.sync.dma_start(wq_s[:], wqt)
            x_tiles = []
            xt0 = xpool.tile([P, KC, SB], dmm, name="xt_tile")
            x_tiles.append(xt0)
            for k in range(KC):
                nc.sync.dma_start(xt0[:, k, :], xt[:, 0, k, :])
            nc.sync.dma_start(wk_s[:], wkt)
            nc.sync.dma_start(wv_s[:], wvt)
            for sb in range(1, NSB):
                xtile = xpool.tile([P, KC, SB], dmm, name="xt_tile")
                x_tiles.append(xtile)
                if sb == 1:
                    nc.sync.dma_start(xtile[:], xt[:, sb, :, :])
                    nc.sync.dma_start(wo_s[:], wot)
                else:
                    nc.sync.dma_start(xtile[:], xt[:, sb, :, :])

            # ones column of v_t (softmax denominator trick)
            ones_sb = cpool.tile([P, NSC * HPC], f32, name="ones_sb")
            nc.gpsimd.memset(ones_sb[:], 1.0)
            nc.vector.tensor_copy(
                out=v_t[:, :, :, DH:DH + 1],
                in_=ones_sb.rearrange("p (a b) -> p a b", b=HPC)[:, :, :, None],
            )

            for sb in range(NSB):
                xt_tile = x_tiles[sb]
                # --- projections for block sb ---
                for w_s, dst in ((wq_s, qt_t), (wk_s, kt_t)):
                    for m in range(MC):
                        pp = pspool.tile([P, SB], f32, name="pp", tag="proj", bufs=2)
                        for k in range(KC):
                            nc.tensor.matmul(
                                pp[:],
                                (w_s[:, k, m * P:(m + 1) * P]),
                                (xt_tile[:, k, :]),
                                start=(k == 0),
                                stop=(k == KC - 1),
                            )
                        nc.vector.tensor_copy(
                            out=dst[:, m, sb * SB:(sb + 1) * SB], in_=pp[:]
                        )
                for sc in range(SB // P):
                    pv = pspool.tile([P, SB], f32, name="pv", tag="proj", bufs=2)
                    for k in range(KC):
                        nc.tensor.matmul(
                            pv[:, :F],
                            (xt_tile[:, k, sc * P:(sc + 1) * P]),
                            (wv_s[:, k, :]),
                            start=(k == 0),
                            stop=(k == KC - 1),
                        )
                    nc.vector.tensor_copy(
                        out=v_t[:, sb * 4 + sc, :, 0:DH],
                        in_=pv[:, :F].rearrange("p (h d) -> p h d", d=DH),
                    )

                # --- attention for q-block qb = sb ---
                qb = sb
                nkv = 4 * (qb + 1)
                for h in range(HPC):
                    prow = (h % MC) * DH
                    mo = h // MC
                    po_t = pspool.tile([DH + 1, SB], f32, name="po_t", tag="o", bufs=2)
                    npair = nkv // 2
                    pts = []
                    for kp in range(npair):
                        psS = pspool.tile([P, 2, SB], f32, name="psS", tag="s", bufs=2)
                        pt = ptpool.tile([P, 2, SB], dmm, name="pt", bufs=3)
                        for idx, kv in ((0, 2 * kp), (1, 2 * kp + 1)):
                            j = kv - 4 * qb   # position relative to diagonal band
                            qlo = P * max(0, j)
                            nc.tensor.matmul(
                                psS[:, idx, qlo:],
                                (kt_t[prow:prow + DH, mo, kv * P:(kv + 1) * P]),
                                (qt_t[prow:prow + DH, mo, qb * SB + qlo:(qb + 1) * SB]),
                                start=True,
                                stop=True,
                            )
                        j0 = 2 * kp - 4 * qb
                        j1 = j0 + 1
                        if j1 <= 0:
                            # fully below the diagonal band: one exp, no mask
                            nc.scalar.activation(
                                pt[:],
                                psS[:],
                                mybir.ActivationFunctionType.Exp,
                                scale=float(1.0 / np.sqrt(DH)),
                            )
                        else:
                            for idx, j in ((0, j0), (1, j1)):
                                qlo = P * max(0, j)
                                nc.scalar.activation(
                                    pt[:, idx, qlo:],
                                    psS[:, idx, qlo:],
                                    mybir.ActivationFunctionType.Exp,
                                    scale=float(1.0 / np.sqrt(DH)),
                                )
                                if j >= 0:
                                    # zero below-diagonal: keep where col >= 128j + row
                                    nc.gpsimd.affine_select(
                                        out=pt[:, idx, :],
                                        in_=pt[:, idx, :],
                                        compare_op=mybir.AluOpType.is_ge,
                                        fill=0.0,
                                        base=-P * j,
                                        pattern=[[1, SB]],
                                        channel_multiplier=-1,
                                    )
                        pts.append(pt)
                        lag = 2 if npair > 2 else 1
                        if kp >= lag:
                            kq = kp - lag
                            for idx, kv in ((0, 2 * kq), (1, 2 * kq + 1)):
                                j = kv - 4 * qb
                                qlo = P * max(0, j)
                                nc.tensor.matmul(
                                    po_t[:, qlo:],
                                    (v_t[:, kv, h, :]),
                                    (pts[kq][:, idx, qlo:]),
                                    start=(kv == 0),
                                    stop=False,
                                )
                    lag = 2 if npair > 2 else 1
                    for kq in range(npair - lag, npair):
                        for idx, kv in ((0, 2 * kq), (1, 2 * kq + 1)):
                            j = kv - 4 * qb
                            qlo = P * max(0, j)
                            nc.tensor.matmul(
                                po_t[:, qlo:],
                                (v_t[:, kv, h, :]),
                                (pts[kq][:, idx, qlo:]),
                                start=(kv == 0),
                                stop=(kv == nkv - 1),
                            )
                    # normalize: out = po * (1/denominator), denominator in row DH
                    dd = spool.tile([1, SB], f32, name="dd", bufs=2)
                    nc.vector.tensor_copy(dd[:], po_t[DH:DH + 1, :])
                    rr = spool.tile([1, SB], f32, name="rr", bufs=2)
                    nc.vector.reciprocal_approx_fast(rr[:], dd[:])
                    rb = spool.tile([DH, SB], f32, name="rb", bufs=2)
                    nc.gpsimd.partition_broadcast(rb[:], rr[:])
                    nc.vector.tensor_mul(
                        ot_t[prow:prow + DH, mo, qb * SB:(qb + 1) * SB],
                        po_t[0:DH, :],
                        rb[:],
                    )
                # --- output projection for this q-block ---
                for sc in range(4 * qb, 4 * qb + 4):
                    for nb in range(ND):
                        py = pspool.tile([P, SB], f32, name="py", tag="proj", bufs=2)
                        for o in range(MC):
                            nc.tensor.matmul(
                                py[:],
                                (ot_t[:, o, sc * P:(sc + 1) * P]),
                                (wo_s[:, o, nb * SB:(nb + 1) * SB]),
                                start=(o == 0),
                                stop=(o == MC - 1),
                            )
                        ys = ypool.tile([P, SB], f16, name="ys", bufs=3)
                        if (sc + nb) % 2 == 0:
                            nc.vector.tensor_copy(ys[:], py[:])
                        else:
                            nc.scalar.copy(ys[:], py[:])
                        nc.sync.dma_start(
                            y[sc * P:(sc + 1) * P, nb * SB:(nb + 1) * SB], ys[:]
                        )

    nc.compile()
    return nc


def _get_nc(s=S):
    if s not in _nc_cache:
        _nc_cache[s] = _build(s)
    return _nc_cache[s]


def make_in_maps(in_features, Wq, Wk, Wv, Wo):
    """Shard full inputs into 8 per-core input dicts (bf16, DMA-linear)."""
    import ml_dtypes
    bf = ml_dtypes.bfloat16
    P = 128
    SB = 512
    NSB = S // SB
    KC = D // P
    MC = F // P
    x = np.asarray(in_features, dtype=np.float32)
    wq = np.asarray(Wq, dtype=np.float32)
    wk = np.asarray(Wk, dtype=np.float32)
    wv = np.asarray(Wv, dtype=np.float32)
    wo = np.asarray(Wo, dtype=np.float32)

    def shuf_w(wt, chunks):  # [D_or_F, cols] -> [P, chunks, cols]
        return np.ascontiguousarray(
            wt.reshape(chunks, P, wt.shape[1]).transpose(1, 0, 2)
        )

    # x[b].T is [D, S]; feature r -> partition r % 128, chunk r // 128.
    xts = []
    for b in range(B):
        xtb = x[b].T.reshape(KC, P, NSB, SB).transpose(1, 2, 0, 3)
        xts.append(np.ascontiguousarray(xtb).astype(bf))

    in_maps = []
    for c in range(NCORES):
        b, g = divmod(c, GROUPS)
        rows = slice(g * F, (g + 1) * F)
        in_maps.append(
            {
                "xt": xts[b],
                "wqt": shuf_w(np.ascontiguousarray(wq[rows, :].T), KC).astype(bf),
                "wkt": shuf_w(np.ascontiguousarray(wk[rows, :].T), KC).astype(bf),
                "wvt": shuf_w(np.ascontiguousarray(wv[rows, :].T), KC).astype(bf),
                "wot": shuf_w(np.ascontiguousarray(wo[:, rows].T), MC).astype(bf),
            }
        )
    return in_maps


def combine_outputs(results):
    """Sum the 4 partial Y per batch element back into [B, S, D]."""
    out = np.zeros((B, S, D), dtype=np.float32)
    for c in range(NCORES):
        b = c // GROUPS
        out[b] += np.asarray(results[c]["y"], dtype=np.float32)
    return out


def kernel(in_features, Wq, Wk, Wv, Wo):
    from concourse import bass_utils

    nc = _get_nc()
    in_maps = make_in_maps(in_features, Wq, Wk, Wv, Wo)
    res = bass_utils.run_bass_kernel_spmd(nc, in_maps, core_ids=list(range(NCORES)))
    return combine_outputs(res.results)
